# revision 75
# baseline (speedup 1.0000x reference)
"""Trainium2 Bass kernel for CrossAttentionFusion (B=4096, D=1024, H=16, L=2).

Math notes (exact algebra, no approximation of the reference graph):
  - nn.MultiheadAttention with seq_len==1: softmax over a single key is
    exactly 1.0, so attention(xq, xkv) == (xkv @ Wv.T + bv) @ Wo.T + bo.
    Q/K projections never affect the output. Fuse W = Wo@Wv host-side.
  - Self-attention + residual: X + X@Wsa.T == X @ (I + Wsa).T, so every
    sa block folds into one matmul; layer 0 additionally folds the input
    projection: Y = vision @ ((I+Wsa0)@vw).T.
  - v/t share all layer weights, so both modalities stack into one
    [2B, D] activation matrix; cross-attention is the same matmul with
    the two row-halves swapped at the residual.
  - LN steps whose output feeds only positively-homogeneous paths (zero
    bias matmuls / relu) into the next LN skip the *rstd row scale:
    LN(c*y) == LN(y) cancels it exactly (steps 1, 2, 4).

Device strategy: pure data-parallel over batch across 8 cores (512 rows
of each modality per core).  Activations live transposed in SBUF
([feature, row], one tile per (feature-chunk, modality-half)) so chained
matmuls need no transposes; weights are pre-transposed host-side into
bf16 lhsT tile images and streamed in 256 KiB blocks.  All matmul
operands are bf16 (PSUM accumulation stays fp32): same 1 cycle/row PE
rate as fp32r in the cost model, but half the DMA traffic, 2x DVE
throughput on the LN applies, and FWL-eligible weight loads on HW.
LayerNorm reduction runs off the PE entirely: per-chunk bf16 add-trees
on the DVE followed by one Pool partition_all_reduce per 512-row slice
(the PE previously spent ~31 us on ones-matmul reductions).  DMA issue
occupancy (~0.81 ns per per-partition byte) is split between the SP and
Activation hardware DGE queues so neither sequencer is co-critical with
the PE; each phase's weight stream rides the queue whose engine is idle
in that phase.  A 12-matmul warmup keeps the PE p-state ramped through
the initial input/weight DMA window, and the fusion head runs fu1 as
two k-phases over 8 live PSUM banks so the final LN latency hides under
t-half matmuls.  The first slice's LN applies interleave chunk-wise into
the second slice's eviction loop on cross-attention ops (keeps the DVE
queue paced so late evicts are not pushed past the PE's last matmul),
and each FFN pre-stages its first two mm1 weight blocks in dedicated
tags.  PSUM evictions carry a scheduler priority boost (offset=16 —
sharp optimum; 24 and 64 regress) so they never queue behind LN work in
the DVE ready-heap: they release the PSUM banks the next op's matmul
groups wait on.  Cost-model time: ~600.2 us (PE busy ~592 us, 98.6%
occupancy; fp32r/bf16 GEMM floor ~587 us).
"""

import numpy as np
from ml_dtypes import bfloat16 as np_bf16

import concourse.bass as bass
import concourse.mybir as mybir
import concourse.tile as tile
from concourse import bacc
from concourse.bass_utils import run_bass_kernel_spmd

H = 16
EPS = 1e-5
D = 1024
DFF = 4 * D
L = 2
B = 4096
NCORES = 8
BLOC = B // NCORES          # rows per modality per core (512)
R = 2 * BLOC                # rows per core (v | t) = 1024
P = 128
KD = D // P                 # 8 feature chunks
KF = DFF // P               # 32 dff chunks
NSL = R // 512              # 2 column slices of 512 rows
F32 = mybir.dt.float32
F32R = mybir.dt.float32r
BF16 = mybir.dt.bfloat16

TRACE = False               # test.py flips this for profiling runs
TRACE_KW = {}

_cache = {}


def _img_lhsT(W):
    """W [dout, din] -> lhsT tile image [128, nm*nk*128], m-major k-minor.

    img[:, (m*nk+k)*128 : +128] == W.T[k*128:(k+1)*128, m*128:(m+1)*128]
    """
    dout, din = W.shape
    nk, nm = din // P, dout // P
    A = np.ascontiguousarray(W.T).reshape(nk, P, nm, P)
    return np.ascontiguousarray(
        A.transpose(1, 2, 0, 3).reshape(P, nm * nk * P)
    ).astype(np.float32).astype(np_bf16)


def _bcol(b):
    """bias vector [dout] -> per-partition tile [128, dout/128]."""
    return np.ascontiguousarray(b.reshape(-1, P).T).astype(np.float32)


def _skipvar_flags(flags):
    (b_cv, b_ct, b_sa1, b_ca0, b_ca1, b_f10, b_f11, b_f20, b_f21,
     b_fu1, b_fu2, ln_nt) = flags
    return {
        0: False,
        1: (not b_f10) and (not b_f20) and (not ln_nt[1]),
        2: (not b_sa1) and (not ln_nt[2]),
        3: False,
        4: (not b_f11) and (not b_f21) and (not ln_nt[4]),
        5: False,
    }


def _build(flags):
    """Build the Bass program. flags: (has_bias..., ln nontrivial...) tuple."""
    (b_cv, b_ct, b_sa1, b_ca0, b_ca1, b_f10, b_f11, b_f20, b_f21,
     b_fu1, b_fu2, ln_nt) = flags
    # ln_nt: 6 bools: nontrivial gain/bias per LN step (per half inside)

    nc = bacc.Bacc("TRN2", target_bir_lowering=False, debug=False)

    # input pre-swizzled host-side: row p holds [k, r] contiguously, so
    # the staging DMAs are max-run-length (cheap to issue)
    din0 = nc.dram_tensor("in0T", [P, KD * R], BF16, kind="ExternalInput")
    wcv = nc.dram_tensor("wcv", [P, KD * KD * P], BF16, kind="ExternalInput")
    wct = nc.dram_tensor("wct", [P, KD * KD * P], BF16, kind="ExternalInput")
    wsa1 = nc.dram_tensor("wsa1", [P, KD * KD * P], BF16, kind="ExternalInput")
    wca = [nc.dram_tensor(f"wca{i}", [P, KD * KD * P], BF16, kind="ExternalInput")
           for i in range(L)]
    wf1 = [nc.dram_tensor(f"wf1_{i}", [P, KD * KF * P], BF16, kind="ExternalInput")
           for i in range(L)]
    wf2 = [nc.dram_tensor(f"wf2_{i}", [P, KF * KD * P], BF16, kind="ExternalInput")
           for i in range(L)]
    wfu1 = nc.dram_tensor("wfu1", [P, 2 * KD * KD * P], BF16, kind="ExternalInput")
    wfu2 = nc.dram_tensor("wfu2", [P, KD * KD * P], BF16, kind="ExternalInput")
    outT = nc.dram_tensor("outT", [D, BLOC], F32, kind="ExternalOutput")

    # optional bias / ln-param DRAM tensors
    def opt(name, shape, cond):
        return nc.dram_tensor(name, shape, F32, kind="ExternalInput") if cond else None

    dbcv = opt("bcv", [P, KD], b_cv)
    dbct = opt("bct", [P, KD], b_ct)
    dbsa1 = opt("bsa1", [P, KD], b_sa1)
    dbca = [opt("bca0", [P, KD], b_ca0), opt("bca1", [P, KD], b_ca1)]
    dbf1 = [opt("bf1_0", [P, KF], b_f10), opt("bf1_1", [P, KF], b_f11)]
    dbf2 = [opt("bf2_0", [P, KD], b_f20), opt("bf2_1", [P, KD], b_f21)]
    dbfu1 = opt("bfu1", [P, KD], b_fu1)
    dbfu2 = opt("bfu2", [P, KD], b_fu2)
    any_ln = any(ln_nt)
    # ln params packed [128, KD*24]: per step s(0..5): [gv, bv, gt, bt] chunks
    dlnp = opt("lnp", [P, KD * 24], any_ln)

    with tile.TileContext(nc) as tc:
        import contextlib
        ctx = contextlib.ExitStack()
        with ctx:
            const = ctx.enter_context(tc.tile_pool(name="const", bufs=1))
            xp = ctx.enter_context(tc.tile_pool(name="xp", bufs=2))
            h1p = ctx.enter_context(tc.tile_pool(name="h1p", bufs=1))
            wbp = ctx.enter_context(tc.tile_pool(name="wbp", bufs=10))
            sqp = ctx.enter_context(tc.tile_pool(name="sqp", bufs=2))
            stp = ctx.enter_context(tc.tile_pool(name="stp", bufs=1))
            bcp = ctx.enter_context(tc.tile_pool(name="bcp", bufs=2))
            outp = ctx.enter_context(tc.tile_pool(name="outp", bufs=4))
            psA = ctx.enter_context(tc.tile_pool(name="psA", bufs=8, space="PSUM"))

            ones = const.tile([P, 2], BF16)
            nc.vector.memset(ones[:, 0:1], -1.0 / D)
            nc.vector.memset(ones[:, 1:2], 1.0 / D)
            eps_t = const.tile([1, 1], F32)
            nc.vector.memset(eps_t[:], EPS)

            # warm the PE (HAM ramp) while the first input/weight DMAs land:
            # long 512-col matmuls keep the PE continuously busy through the
            # DMA window so the first real matmul runs at full p-state.
            warm = const.tile([P, 512], BF16)
            nc.vector.memset(warm[:], 0.0)
            wps0 = psA.tile([1, 2], F32, tag="mm", name="warm0")
            for _ in range(6):
                nc.tensor.matmul(wps0[:], lhsT=ones[:, 0:1], rhs=ones[:, 0:2],
                                 start=True, stop=True)
            wps = psA.tile([1, 512], F32, tag="mm", name="warm")
            for _ in range(10):
                nc.tensor.matmul(wps[:], lhsT=ones[:, 0:1], rhs=warm[:],
                                 start=True, stop=True)

            def load_bias(dram):
                if dram is None:
                    return None
                t = const.tile([P, dram.shape[1]], F32, tag=dram.name)
                nc.sync.dma_start(t[:], dram[:])
                return t

            tbcv = load_bias(dbcv)
            tbct = load_bias(dbct)
            tbsa1 = load_bias(dbsa1)
            tbca = [load_bias(d) for d in dbca]
            tbf1 = [load_bias(d) for d in dbf1]
            tbf2 = [load_bias(d) for d in dbf2]
            tbfu1 = load_bias(dbfu1)
            tbfu2 = load_bias(dbfu2)
            tlnp = load_bias(dlnp)

            AT = mybir.AluOpType
            skipvar = _skipvar_flags(flags)
            # DMA issue-queue occupancy scales with bytes (~0.81 ns/B
            # per partition) and only SP + Activation have hardware DGE.
            # Policy: each phase's weight stream goes on the queue whose
            # ENGINE is idle during that phase (Act is idle during ca/mm2,
            # busy with relu/copy evicts during proj/sa/mm1/fu), so issue
            # slices never delay eviction dispatch.
            dmaq = (nc.sync, nc.scalar)

            def new_gen(name):
                # X[k][h]: feature-chunk k, half h — separate tiles per half
                # so cross-half LN applies never falsely serialize matmuls.
                return [{h: xp.tile([P, BLOC], BF16, tag=f"x{k}_{h}",
                                    name=f"{name}{k}_{h}")
                         for h in range(NSL)} for k in range(KD)]

            import concourse.bass_isa as bass_isa

            def ln_begin(step, n, tree_pool=False):
                """Start LN state for one 512-row slice of step's output.
                tree_pool: run the mean add-tree on gpsimd (used for the
                deferred slice of skip steps, whose stats are
                latency-tolerant, to relieve DVE at op boundaries)."""
                return {"step": step, "n": n, "skip": skipvar[step],
                        "tpool": tree_pool and skipvar[step]}

            def ln_chunk(st, Y, k, sq_dve=False):
                """Fold chunk k of the producing op into the LN reduction.
                Chunk sums accumulate on the DVE (bf16 add-tree) so the PE
                spends zero cycles on LN; the cross-partition reduce happens
                once per slice on Pool in ln_stats."""
                step, n = st["step"], st["n"]
                yk = Y[k][n]
                teng = nc.gpsimd if st.get("tpool") else nc.vector
                if k == 0:
                    st["macc"] = yk          # tree materializes at k==1
                else:
                    if k == 1:
                        macc = sqp.tile([P, 512], BF16, tag="macc",
                                        name=f"ma{step}{n}")
                        teng.tensor_tensor(macc[:], st["macc"][:], yk[:],
                                           op=AT.add)
                        st["macc"] = macc
                    else:
                        teng.tensor_tensor(st["macc"][:], st["macc"][:],
                                           yk[:], op=AT.add)
                if not st["skip"]:
                    sq = sqp.tile([P, 512], BF16, tag="sq", name=f"sq{step}{n}{k}")
                    if sq_dve:
                        nc.vector.tensor_tensor(sq[:], yk[:], yk[:], op=AT.mult)
                    else:
                        nc.scalar.activation(sq[:], yk[:],
                                             mybir.ActivationFunctionType.Square)
                    if k == 0:
                        st["qacc"] = sq
                    elif k == 1:
                        qacc = sqp.tile([P, 512], BF16, tag="qacc",
                                        name=f"qa{step}{n}")
                        nc.vector.tensor_tensor(qacc[:], st["qacc"][:], sq[:],
                                                op=AT.add)
                        st["qacc"] = qacc
                    else:
                        nc.vector.tensor_tensor(st["qacc"][:], st["qacc"][:],
                                                sq[:], op=AT.add)

            def ln_stats(st, Y):
                """Cross-partition reduce (Pool) + stats + broadcasts."""
                step, n = st["step"], st["n"]
                skip = st["skip"]
                parm = sqp.tile([P, 512], F32, tag="parm", name=f"pm{step}{n}")
                nc.gpsimd.partition_all_reduce(parm[:], st["macc"][:],
                                               channels=P,
                                               reduce_op=bass_isa.ReduceOp.add)
                nm = stp.tile([1, 512], BF16, tag="nm", name=f"nm{step}{n}")
                nc.scalar.mul(nm[:], parm[0:1, :], -1.0 / D)
                if not skip:
                    parq = sqp.tile([P, 512], F32, tag="parq", name=f"pq{step}{n}")
                    nc.gpsimd.partition_all_reduce(parq[:], st["qacc"][:],
                                                   channels=P,
                                                   reduce_op=bass_isa.ReduceOp.add)
                    qv = stp.tile([1, 512], F32, tag="qv", name=f"qv{step}{n}")
                    nc.scalar.mul(qv[:], parq[0:1, :], 1.0 / D)
                    t1 = stp.tile([1, 512], F32, tag="t1", name=f"t1_{step}{n}")
                    var = stp.tile([1, 512], F32, tag="var", name=f"var{step}{n}")
                    rstd = stp.tile([1, 512], BF16, tag="rstd", name=f"rstd{step}{n}")
                    cc = stp.tile([1, 512], BF16, tag="cc", name=f"cc{step}{n}")
                    nc.vector.tensor_tensor(t1[:], nm[:], nm[:], op=AT.mult)
                    nc.vector.tensor_tensor(var[:], qv[:], t1[:], op=AT.subtract)
                    nc.scalar.activation(var[:], var[:],
                                         mybir.ActivationFunctionType.Sqrt,
                                         bias=eps_t[:], scale=1.0)
                    with nc.allow_low_precision(
                            reason="bf16 rstd: row-scale error cancels "
                                   "through the next LN (homogeneity)"):
                        nc.vector.reciprocal(rstd[:], var[:])
                    nc.vector.tensor_tensor(cc[:], nm[:], rstd[:], op=AT.mult)
                    rb = bcp.tile([P, 512], BF16, tag="rb", name=f"rb{step}{n}")
                    cb = bcp.tile([P, 512], BF16, tag="cb", name=f"cb{step}{n}")
                    nc.gpsimd.partition_broadcast(rb[:], rstd[:])
                    nc.gpsimd.partition_broadcast(cb[:], cc[:])
                    st["rb"], st["cb"] = rb, cb
                else:
                    cb = bcp.tile([P, 512], BF16, tag="cb", name=f"cb{step}{n}")
                    nc.gpsimd.partition_broadcast(cb[:], nm[:])
                    st["cb"] = cb

            def ln_apply(st, Y):
                """In-place apply, split DVE/GpSimd (per-half g/b flagged)."""
                step, n = st["step"], st["n"]
                nontriv = ln_nt[step]
                skip = st["skip"]
                for k in range(KD):
                    yk = Y[k][n]
                    # balance: DVE ~327 ns/op vs Pool ~1110 ns/op
                    eng = nc.vector if k < 6 else nc.gpsimd
                    if not skip:
                        eng.tensor_tensor(yk[:], yk[:],
                                          st["rb"][:], op=AT.mult)
                        eng.tensor_tensor(yk[:], yk[:],
                                          st["cb"][:], op=AT.add)
                    else:
                        eng.tensor_tensor(yk[:], yk[:],
                                          st["cb"][:], op=AT.add)
                    if nontriv:
                        # per-half gain/bias: n==0 -> v params, n==1 -> t
                        base = step * 4 * KD + (0 if n == 0 else 2 * KD)
                        g = tlnp[:, base + k:base + k + 1]
                        bb = tlnp[:, base + KD + k:base + KD + k + 1]
                        nc.vector.tensor_scalar(yk[:], in0=yk[:],
                                                scalar1=g, scalar2=bb,
                                                op0=AT.mult, op1=AT.add)

            def ln_apply_one(st, Y, k):
                """Apply LN to a single chunk — used to interleave the
                inline slice's applies into the other slice's eviction loop
                so they don't block the late evicts in the DVE queue."""
                step, n = st["step"], st["n"]
                yk = Y[k][n]
                eng = nc.vector if k < 6 else nc.gpsimd
                if not st["skip"]:
                    eng.tensor_tensor(yk[:], yk[:], st["rb"][:], op=AT.mult)
                    eng.tensor_tensor(yk[:], yk[:], st["cb"][:], op=AT.add)
                else:
                    eng.tensor_tensor(yk[:], yk[:], st["cb"][:], op=AT.add)
                if ln_nt[step]:
                    base = step * 4 * KD + (0 if n == 0 else 2 * KD)
                    g = tlnp[:, base + k:base + k + 1]
                    bb = tlnp[:, base + KD + k:base + KD + k + 1]
                    nc.vector.tensor_scalar(yk[:], in0=yk[:],
                                            scalar1=g, scalar2=bb,
                                            op0=AT.mult, op1=AT.add)

            def ln_finish(st, Y):
                ln_stats(st, Y)
                ln_apply(st, Y)

            def evict(kind, Yo, m, on, ps, bt, X=None):
                bias = bt[:, m:m + 1] if bt is not None else 0.0
                # evicts release PSUM banks the next op's matmul groups wait
                # on: boost them past earlier-emitted applies/trees in the
                # scheduler's ready-heap so they never queue behind LN work
                with tc.high_priority(offset=16):
                    if kind == "res":
                        nc.vector.scalar_tensor_tensor(
                            Yo[m][on][:], in0=ps[:], scalar=bias,
                            in1=X[m][on][:], op0=AT.add, op1=AT.add)
                    elif bt is not None:
                        nc.vector.tensor_scalar_add(Yo[m][on][:], in0=ps[:],
                                                    scalar1=bias)
                    else:
                        nc.scalar.activation(Yo[m][on][:], ps[:],
                                             mybir.ActivationFunctionType.Copy)

            def linear_dd(X, wimg, bt, kind, swap=False, Ynew=None, name="",
                          ln_step=None, nlist=None, carry_in=None,
                          defer_out=False, wq=None):
                """[D x D] matmul over resident X; kind: 'copy' (sa: psum->Y)
                or 'res' (ca: Y = X_other_half + psum).  swap: cross halves.
                n-outer.  carry_in: deferred LN applies from the previous op,
                flushed after this op's second eviction (so they sit behind
                only two evicts in the DVE queue).  defer_out: leave the last
                slice's LN apply to the next op (stats still run inline)."""
                Yo = Ynew
                if nlist is None:
                    nlist = (1, 0) if swap else (0, 1)
                carry = list(carry_in or [])
                out_carry = []
                inline_ap = None
                for ni, n in enumerate(nlist):
                    on = (1 - n) if swap else n
                    st = (ln_begin(ln_step, on, tree_pool=(ni == 1))
                          if ln_step is not None else None)
                    if ni == 0 and kind == "res":
                        # res evicts read the deferred half as residual from
                        # eviction 0 on — flush before any eviction.
                        for cst, cy in carry:
                            ln_apply(cst, cy)
                        carry = []
                    for m in range(KD):
                        wt = wbp.tile([P, KD * P], BF16, tag="w", bufs=8,
                                      name=f"w{name}{m}{n}")
                        (wq or nc.sync).dma_start(
                            wt[:], wimg[:, m * KD * P:(m + 1) * KD * P])
                        ps = psA.tile([P, 512], F32, tag="mm",
                                      name=f"p{name}{m}{n}")
                        for k in range(KD):
                            nc.tensor.matmul(
                                ps[:], lhsT=wt[:, k * P:(k + 1) * P],
                                rhs=X[k][n][:], start=(k == 0),
                                stop=(k == KD - 1))
                        evict(kind, Yo, m, on, ps, bt, X)
                        if ni == 0 and m == 1 and carry:
                            for cst, cy in carry:
                                ln_apply(cst, cy)
                            carry = []
                        if st is not None:
                            ln_chunk(st, Yo, m, sq_dve=(kind == "copy"))
                        if inline_ap is not None:
                            ln_apply_one(inline_ap, Yo, m)
                    if st is not None:
                        ln_stats(st, Yo)
                        if ni == len(nlist) - 1 and defer_out:
                            out_carry.append((st, Yo))
                        elif ni == 0 and kind == "res":
                            # interleave this slice's applies chunk-wise into
                            # the next slice's eviction loop: keeps the DVE
                            # queue paced so late evicts (which recycle PSUM
                            # banks for the next op) aren't pushed past the
                            # PE's last matmul.  ("copy" ops evict on Act —
                            # no DVE congestion, and the next op wants these
                            # applies as early as possible.)
                            inline_ap = st
                        else:
                            ln_apply(st, Yo)
                return out_carry

            def ffn(X, li, ln_step=None, nlist=(0, 1), carry_in=None,
                    defer_out=False):
                """relu(X@fw1.T+b1)@fw2.T+b2 with residual into new Y tiles."""
                Ynew = new_gen(f"yf{li}")
                carry = list(carry_in or [])
                out_carry = []
                # pre-stage mm1's first two weight blocks in dedicated tags:
                # their DMAs issue right after the previous op's stream with
                # no tag-rotation WAR, so mm1's first groups aren't gated on
                # a just-in-time transfer (+sem) at the op boundary.  Reused
                # by both slices.
                staged = []
                for sm in range(2):
                    wt = wbp.tile([P, KD * P], BF16, tag=f"f1h{li}{sm}",
                                  bufs=1, name=f"wf1s_{li}{sm}")
                    nc.sync.dma_start(
                        wt[:], wf1[li][:, sm * KD * P:(sm + 1) * KD * P])
                    staged.append(wt)
                for ni, n in enumerate(nlist):
                    st = (ln_begin(ln_step, n, tree_pool=(ni == 1))
                          if ln_step is not None else None)
                    h1 = []
                    for m in range(KF):
                        if m < 2:
                            wt = staged[m]
                        else:
                            wt = wbp.tile([P, KD * P], BF16, tag="w", bufs=8,
                                          name=f"wf1_{li}{n}{m}")
                            nc.sync.dma_start(
                                wt[:], wf1[li][:, m * KD * P:(m + 1) * KD * P])
                        ps = psA.tile([P, 512], F32, tag="mm",
                                      name=f"pf1_{li}{n}{m}")
                        for k in range(KD):
                            nc.tensor.matmul(
                                ps[:], lhsT=wt[:, k * P:(k + 1) * P],
                                rhs=X[k][n][:], start=(k == 0),
                                stop=(k == KD - 1))
                        ht = h1p.tile([P, 512], BF16, tag=f"h{m}",
                                      name=f"h{li}{n}{m}")
                        bias = (tbf1[li][:, m:m + 1]
                                if tbf1[li] is not None else 0.0)
                        nc.scalar.activation(
                            ht[:], ps[:], mybir.ActivationFunctionType.Relu,
                            bias=bias)
                        h1.append(ht)
                        if ni == 0 and m == 1:
                            for cst, cy in carry:
                                ln_apply(cst, cy)
                            carry = []
                    for m in range(KD):      # mm2: two 1 MiB half-blocks per m
                        ps = psA.tile([P, 512], F32, tag="mm", name=f"pf2_{li}{n}{m}")
                        for kb in range(2):
                            wt = wbp.tile([P, 16 * P], BF16, tag="w", bufs=8,
                                          name=f"wf2_{li}{n}{m}{kb}")
                            off = (m * KF + kb * 16) * P
                            nc.scalar.dma_start(
                                wt[:], wf2[li][:, off:off + 16 * P])
                            for k in range(16):
                                kk = kb * 16 + k
                                nc.tensor.matmul(ps[:], lhsT=wt[:, k * P:(k + 1) * P],
                                                 rhs=h1[kk][:], start=(kk == 0),
                                                 stop=(kk == KF - 1))
                        bias = tbf2[li][:, m:m + 1] if tbf2[li] is not None else 0.0
                        with tc.high_priority(offset=16):
                            nc.vector.scalar_tensor_tensor(
                                Ynew[m][n][:], in0=ps[:], scalar=bias,
                                in1=X[m][n][:], op0=AT.add, op1=AT.add)
                        if st is not None:
                            ln_chunk(st, Ynew, m)
                    if st is not None:
                        ln_stats(st, Ynew)
                        if ni == len(nlist) - 1 and defer_out:
                            out_carry.append((st, Ynew))
                        else:
                            ln_apply(st, Ynew)
                return Ynew, out_carry

            # ---------------- layer 0 fused input-proj + self-attn ----------
            # Y[:, v] = vision @ Wcv.T (+bcv); Y[:, t] = text @ Wct.T (+bct)
            # t half first so its LN hides under the v half's matmuls and
            # ca0 (which consumes t rows first) can start immediately.
            # Input staged as 4 contiguous pieces on the Activation DGE
            # (done issuing before its engine's first evict): ~6 us of issue
            # vs ~17 us for the strided 3D form, so the proj weight stream
            # on SP is never backlogged.
            xpcs = []
            for i in range(4):
                xt = wbp.tile([P, 2 * R], BF16, tag=f"xpc{i}", bufs=1,
                              name=f"xpc{i}")
                nc.scalar.dma_start(xt[:], din0[:, i * 2 * R:(i + 1) * 2 * R])
                xpcs.append(xt)
            Y = new_gen("y0")
            for half, (wimg, bt) in ((1, (wct, tbct)), (0, (wcv, tbcv))):
                st = ln_begin(0, half)
                for m in range(KD):
                    wt = wbp.tile([P, KD * P], BF16, tag="w", bufs=8,
                                  name=f"w0_{half}_{m}")
                    nc.sync.dma_start(
                        wt[:], wimg[:, m * KD * P:(m + 1) * KD * P])
                    ps = psA.tile([P, BLOC], F32, tag="mm",
                                  name=f"p0_{half}_{m}")
                    for k in range(KD):
                        off = (k % 2) * R + half * BLOC
                        nc.tensor.matmul(
                            ps[:], lhsT=wt[:, k * P:(k + 1) * P],
                            rhs=xpcs[k // 2][:, off:off + BLOC],
                            start=(k == 0), stop=(k == KD - 1))
                    evict("copy", Y, m, half, ps, bt)
                    ln_chunk(st, Y, m, sq_dve=True)
                ln_stats(st, Y)
                if half == 1:
                    ln_apply(st, Y)      # t half: hidden under v half's work
                else:
                    carry0 = [(st, Y)]   # v half: deferred into ca0

            # ---------------- layers (unrolled) ----------
            # Deferral chain: each op's last-slice LN apply is emitted inside
            # the NEXT op (after its second eviction), so the applies overlap
            # that op's matmuls instead of serializing the DVE at boundaries.
            X = Y
            Yc = new_gen("yc0")
            carry = linear_dd(X, wca[0], tbca[0], "res", swap=True, Ynew=Yc,
                              name="ca0", ln_step=1, nlist=(1, 0),
                              carry_in=carry0, defer_out=True, wq=nc.scalar)
            X = Yc
            X, carry = ffn(X, 0, ln_step=2, nlist=(0, 1), carry_in=carry,
                           defer_out=True)

            Ys = new_gen("ys1")
            carry = linear_dd(X, wsa1, tbsa1, "copy", Ynew=Ys, name="sa1",
                              ln_step=3, nlist=(0, 1), carry_in=carry,
                              defer_out=True)
            X = Ys
            Yc = new_gen("yc1")
            carry = linear_dd(X, wca[1], tbca[1], "res", swap=True, Ynew=Yc,
                              name="ca1", ln_step=4, nlist=(0, 1),
                              carry_in=carry, defer_out=True, wq=nc.scalar)
            X = Yc
            # first slice must be one whose LN is already applied: ca1's
            # inline slice is 1 (rhs 0 -> swap), deferred is 0 -> go (1, 0).
            X, carry = ffn(X, 1, ln_step=5, nlist=(1, 0), carry_in=carry,
                           defer_out=False)
            assert not carry

            # ---------------- fusion head ----------
            # Two-phase fu1: phase 1 contracts the t half for ALL 8 output
            # chunks (8 live PSUM groups, ~14 us of PE work) so the v half's
            # final LN stats+applies hide completely behind it; phase 2
            # finishes each group with the v chunks and relu-evicts.
            fu1w = []
            for mb in range(8):
                wt = wbp.tile([P, 2 * KD * P], BF16, tag=f"wfu1_{mb}",
                              bufs=1, name=f"wfu1_{mb}")
                nc.sync.dma_start(
                    wt[:], wfu1[:, mb * 2 * KD * P:(mb + 1) * 2 * KD * P])
                fu1w.append(wt)
            fu2w = []
            for mb in range(4):
                wt = wbp.tile([P, 2 * KD * P], BF16, tag=f"wfu2_{mb}",
                              bufs=1, name=f"wfu2_{mb}")
                nc.sync.dma_start(
                    wt[:], wfu2[:, mb * 2 * KD * P:(mb + 1) * 2 * KD * P])
                fu2w.append(wt)
            pfu = []
            for m in range(8):
                ps = psA.tile([P, 512], F32, tag="mm", name=f"pfu1_{m}")
                pfu.append(ps)
                for j, k in enumerate(range(KD, 2 * KD)):    # t chunks
                    nc.tensor.matmul(
                        ps[:], lhsT=fu1w[m][:, k * P:(k + 1) * P],
                        rhs=X[k - KD][1][:], start=(j == 0), stop=False)
            hf = []
            for m in range(8):
                ps = pfu[m]
                for k in range(KD):                          # v chunks
                    nc.tensor.matmul(
                        ps[:], lhsT=fu1w[m][:, k * P:(k + 1) * P],
                        rhs=X[k][0][:], start=False, stop=(k == KD - 1))
                ht = h1p.tile([P, 512], BF16, tag=f"h{m}", name=f"hf{m}")
                bias = tbfu1[:, m:m + 1] if tbfu1 is not None else 0.0
                nc.scalar.activation(ht[:], ps[:],
                                     mybir.ActivationFunctionType.Relu,
                                     bias=bias)
                hf.append(ht)
            for m in range(8):
                wt = fu2w[m // 2]
                mi = m % 2
                # last chunk in two 256-col groups so its evict+DMA overlap
                # the remaining matmuls instead of serializing at the drain
                cols = (((0, 256), (256, 384), (384, 512))
                        if m == 7 else ((0, 512),))
                for ci, (c0, c1) in enumerate(cols):
                    ps = psA.tile([P, c1 - c0], F32, tag="mm",
                                  name=f"pfu2_{m}_{c0}")
                    for k in range(KD):
                        nc.tensor.matmul(
                            ps[:],
                            lhsT=wt[:, (mi * KD + k) * P:(mi * KD + k + 1) * P],
                            rhs=hf[k][:, c0:c1], start=(k == 0),
                            stop=(k == KD - 1))
                    ot = outp.tile([P, c1 - c0], F32, tag="o",
                                   name=f"o{m}_{c0}")
                    if tbfu2 is not None:
                        nc.vector.tensor_scalar_add(ot[:], in0=ps[:],
                                                    scalar1=tbfu2[:, m:m + 1])
                    else:
                        nc.scalar.activation(ot[:], ps[:],
                                             mybir.ActivationFunctionType.Copy)
                    dmaq[(m + ci) % 2].dma_start(
                        outT[m * P:(m + 1) * P, c0:c1], ot[:])

    nc.compile()
    return nc


def _prep(inputs):
    """Host-side weight fusion + lhsT image construction (float64 math)."""
    g = {k: np.asarray(v, dtype=np.float64) for k, v in inputs.items()}
    I = np.eye(D)

    def att_fuse(wqkv, bqkv, wo, bo):
        wv = wqkv[2 * D:]
        bv = bqkv[2 * D:]
        return wo @ wv, wo @ bv + bo

    Wsa, bsa, Wca, bca = [], [], [], []
    for i in range(L):
        w, b = att_fuse(g["sa_wqkv"][i], g["sa_bqkv"][i], g["sa_wo"][i], g["sa_bo"][i])
        Wsa.append(w); bsa.append(b)
        w, b = att_fuse(g["ca_wqkv"][i], g["ca_bqkv"][i], g["ca_wo"][i], g["ca_bo"][i])
        Wca.append(w); bca.append(b)

    M0 = I + Wsa[0]
    Wcv, Wct = M0 @ g["vw"], M0 @ g["tw"]
    bcv = M0 @ g["vb"] + bsa[0]
    bct = M0 @ g["tb"] + bsa[0]
    Wsa1 = I + Wsa[1]

    weights = {
        "wcv": _img_lhsT(Wcv), "wct": _img_lhsT(Wct), "wsa1": _img_lhsT(Wsa1),
        "wca0": _img_lhsT(Wca[0]), "wca1": _img_lhsT(Wca[1]),
        "wf1_0": _img_lhsT(g["fw1"][0]), "wf1_1": _img_lhsT(g["fw1"][1]),
        "wf2_0": _img_lhsT(g["fw2"][0]), "wf2_1": _img_lhsT(g["fw2"][1]),
        "wfu1": _img_lhsT(g["fus_w1"]), "wfu2": _img_lhsT(g["fus_w2"]),
    }

    def nz(x):
        return bool(np.any(x != 0.0))

    biases = {
        "bcv": bcv, "bct": bct, "bsa1": bsa[1], "bca0": bca[0], "bca1": bca[1],
        "bf1_0": g["fb1"][0], "bf1_1": g["fb1"][1],
        "bf2_0": g["fb2"][0], "bf2_1": g["fb2"][1],
        "bfu1": g["fus_b1"], "bfu2": g["fus_b2"],
    }
    bflags = []
    for name in ("bcv", "bct", "bsa1", "bca0", "bca1", "bf1_0", "bf1_1",
                 "bf2_0", "bf2_1", "bfu1", "bfu2"):
        has = nz(biases[name])
        bflags.append(has)
        if has:
            weights[name] = _bcol(biases[name])

    # LN params per step: (l0:ln1, l0:ln2/3, l0:ln2/3, l1:ln1, l1:ln2/3,
    # l1:ln2/3); v-half params then t-half params.
    ln_steps = []
    for i in range(L):
        ln_steps.append((g["ln1g"][i], g["ln1b"][i], g["ln1g"][i], g["ln1b"][i]))
        ln_steps.append((g["ln2g"][i], g["ln2b"][i], g["ln3g"][i], g["ln3b"][i]))
        ln_steps.append((g["ln2g"][i], g["ln2b"][i], g["ln3g"][i], g["ln3b"][i]))
    ln_nt = tuple(
        not (np.all(gv == 1) and np.all(bv == 0) and np.all(gt == 1) and np.all(bt == 0))
        for (gv, bv, gt, bt) in ln_steps
    )
    if any(ln_nt):
        cols = []
        for (gv, bv, gt, bt) in ln_steps:
            cols += [_bcol(gv), _bcol(bv), _bcol(gt), _bcol(bt)]
        weights["lnp"] = np.concatenate(cols, axis=1)

    flags = tuple(bflags) + (ln_nt,)
    return weights, flags


def sim_time_ns():
    """Cost-model exec-time estimate of the last-built program (used by
    test.py when the axon NTFF trace path is unavailable)."""
    if not _cache:
        return None
    nc = next(iter(_cache.values()))
    from concourse.timeline_sim import TimelineSim
    return TimelineSim(nc, trace=False).simulate()


def kernel(**inputs):
    vision = np.ascontiguousarray(np.asarray(inputs["vision_features"], np.float32))
    text = np.ascontiguousarray(np.asarray(inputs["text_features"], np.float32))

    weights, flags = _prep(inputs)
    if flags not in _cache:
        _cache[flags] = _build(flags)
    nc = _cache[flags]

    in_maps = []
    for c in range(NCORES):
        rs = slice(c * BLOC, (c + 1) * BLOC)
        in0 = np.concatenate([
            np.ascontiguousarray(vision[rs].T),
            np.ascontiguousarray(text[rs].T),
        ], axis=1).astype(np_bf16)
        # swizzle [D, R] -> [P, KD*R]: row p holds the 8 k-chunks contiguously
        in0 = np.ascontiguousarray(
            in0.reshape(KD, P, R).transpose(1, 0, 2).reshape(P, KD * R))
        m = dict(weights)
        m["in0T"] = in0
        in_maps.append(m)

    res = run_bass_kernel_spmd(nc, in_maps, core_ids=list(range(NCORES)),
                               trace=TRACE, **TRACE_KW)
    kernel.last_result = res

    out = np.empty((B, D), dtype=np.float32)
    for c in range(NCORES):
        out[c * BLOC:(c + 1) * BLOC, :] = res.results[c]["outT"].T
    return out

